# revision 1
# baseline (speedup 1.0000x reference)
"""Trainium2 Bass kernel for nn_Explore_Decoder (scatter_memory).

Full computation:
    a      = all_memory @ U_w                         [B,S,H]
    l      = (last_memory @ W_w)[:,None,:]            [B,1,H]
    scores = (tanh(a+l) @ V_w + V_b)[...,0]           [B,S]
    scores = where(mask, -1e9, scores)
    alpha  = softmax(scores, axis=1)
    out_e  = sum(alpha * all_memory, axis=1)          [B,H]
    feats  = concat([out_e, last_memory], axis=1)     [B,2H]
    logits = feats @ E_w                              [B,N]
    logits = where(seen_item, -inf, logits)           (scatter of item_seq)
    return sigmoid(logits)

Sharding (8 cores):
  Phase 1 (attention): data-parallel over B; core c owns rows [c*128,(c+1)*128).
    Produces featsT [2H, 128] per core, AllGather -> featsT for all B on every core.
  Phase 2 (logits): tensor-parallel over N; core c owns cols [c*6250,(c+1)*6250).
    Dense: out = sigmoid(featsT.T @ E_w[:, cols_c]) for all 1024 rows.
    Scatter: host-precomputed flat offsets of seen items in the local column
    range; indirect DMA writes 0.0 ( = sigmoid(-inf)) over the dense output.
"""

import numpy as np

B, S, H, N = 1024, 100, 128, 50000
NCORES = 8
BL = B // NCORES          # 128 batch rows per core (phase 1)
NL = N // NCORES          # 6250 vocab cols per core (phase 2)
H2 = 2 * H
SB = 4                    # s-values per phase-1 block
NSB = S // SB             # 25 blocks
SENTINEL = np.int32(2**30)

_BUILT = {}               # scat_w -> compiled Bass module
_LAST_RESULTS = None      # BassKernelResults of the most recent run (for tests)


def _default_spec(n_scat=100):
    # representative scatter spec for profiling: uniform item distribution
    per = 12800 / NCORES
    dep = tuple(min(NCORES - 1, int((w + 1) * 128 / per)) for w in range(n_scat))
    return (n_scat, dep)


def _build(scat_w, reps: int = 1, timeline: bool = False,
           no_scatter: bool = False, two_mm: bool = False,
           use_f32r: bool = False):
    # scat_w: (n_insts, dep_cb tuple) from host packing
    import concourse.bass as bass
    import concourse.mybir as mybir
    import concourse.tile as tile
    from concourse import bacc
    from concourse.masks import make_identity

    f32 = mybir.dt.float32
    bf16 = mybir.dt.bfloat16
    i32 = mybir.dt.int32
    AF = mybir.ActivationFunctionType
    ALU = mybir.AluOpType
    AX = mybir.AxisListType

    nc = bacc.Bacc(None, target_bir_lowering=False, debug=False)

    am = nc.dram_tensor("am", [BL, S * H], f32, kind="ExternalInput")
    lm = nc.dram_tensor("lm", [BL, H], f32, kind="ExternalInput")
    maskb = nc.dram_tensor("maskb", [BL, S], f32, kind="ExternalInput")
    uw = nc.dram_tensor("uw", [H, H], bf16, kind="ExternalInput")
    ww = nc.dram_tensor("ww", [H, H], bf16, kind="ExternalInput")
    vw = nc.dram_tensor("vw", [H, 1], bf16, kind="ExternalInput")
    ew = nc.dram_tensor("ew", [H2, NL], bf16, kind="ExternalInput")
    ewl = nc.dram_tensor("ewl", [H2, NL], bf16, kind="ExternalInput")
    f32r = mybir.dt.float32r
    ewf = (nc.dram_tensor("ewf", [H2, NL], f32r, kind="ExternalInput")
           if use_f32r else None)
    n_scat, dep_cb = scat_w
    # [p, w] = offset for partition p of indirect-DMA instruction w;
    # instruction w may fire once the store of block dep_cb[w] has landed
    sidx = nc.dram_tensor("sidx", [128, n_scat], i32, kind="ExternalInput")
    out = nc.dram_tensor("out", [B, NL], f32, kind="ExternalOutput")
    # timeline mode: single-core cost-model sim can't price collectives or
    # full-tensor indirect APs; swap in traffic-equivalent stand-ins
    dumout = nc.dram_tensor("dumout", [128, 1], f32) if timeline else None

    with tile.TileContext(nc) as tc:
      for _rep in range(reps):
        with (
            tc.tile_pool(name="consts", bufs=1) as cp,
            tc.tile_pool(name="amp", bufs=1) as amp,
            tc.tile_pool(name="ewp", bufs=1) as ewp,
            tc.tile_pool(name="dram", bufs=1, space="DRAM") as dp,
            tc.tile_pool(name="smax", bufs=1) as sm,
        ):
            ident = cp.tile([128, 128], f32)
            make_identity(nc, ident[:])
            uw_sb = cp.tile([H, H], bf16)
            nc.sync.dma_start(out=uw_sb[:], in_=uw[:, :])
            ww_sb = cp.tile([H, H], bf16)
            nc.sync.dma_start(out=ww_sb[:], in_=ww[:, :])
            vw_sb = cp.tile([H, 1], bf16)
            nc.sync.dma_start(out=vw_sb[:], in_=vw[:, :])
            maskb_sb = cp.tile([BL, S], f32)
            nc.sync.dma_start(out=maskb_sb[:], in_=maskb[:, :])
            lm_sb = cp.tile([BL, H], f32)
            nc.sync.dma_start(out=lm_sb[:], in_=lm[:, :])

            am_t = amp.tile([BL, S * H], f32)
            AMCH = 20 * H
            for a0 in range(0, S * H, AMCH):
                nc.sync.dma_start(out=am_t[:, a0:a0 + AMCH],
                                  in_=am[:, a0:a0 + AMCH])
            amv = am_t[:].rearrange("p (s h) -> p s h", h=H)

            feats_local = dp.tile([H2, BL], f32)
            gath = dp.tile([NCORES * H2, BL], f32)

            # ---------------- Phase 1: attention over S, rows of this core ----
            with (
                tc.tile_pool(name="ps_t", bufs=3, space="PSUM") as ps_t,
                tc.tile_pool(name="ps_z", bufs=2, space="PSUM") as ps_z,
                tc.tile_pool(name="ps_acc", bufs=1, space="PSUM") as ps_acc,
                tc.tile_pool(name="xtp", bufs=6) as xtp,
                tc.tile_pool(name="tzp", bufs=6) as tzp,
            ):
                # last_memory^T  [H, BL] , replicated x SB for the Z matmul rhs
                lmT_ps = ps_t.tile([128, 512], f32, tag="tps")
                nc.tensor.transpose(out=lmT_ps[:, :H], in_=lm_sb[:],
                                    identity=ident[:])
                lmT_sb = cp.tile([H, BL], f32)
                nc.vector.tensor_copy(lmT_sb[:], lmT_ps[:, :H])
                lmT_rep = cp.tile([H, SB * BL], bf16)
                nc.vector.tensor_copy(
                    lmT_rep[:].rearrange("h (s b) -> h s b", s=SB),
                    lmT_sb[:].unsqueeze(1).broadcast_to([H, SB, BL]),
                )
                # feats rows H..2H = last_memory^T (raw)
                nc.sync.dma_start(out=feats_local[H:H2, :], in_=lmT_sb[:])

                sc_ps = ps_acc.tile([BL, S], f32, tag="sc")
                for sb in range(NSB):
                    xt_ps = ps_t.tile([128, SB * 128], f32, tag="tps")
                    for j in range(SB):
                        s = sb * SB + j
                        nc.tensor.transpose(
                            out=xt_ps[:, j * 128:(j + 1) * 128],
                            in_=amv[:, s, :],
                            identity=ident[:],
                        )
                    xt = xtp.tile([128, SB * 128], bf16)
                    if sb % 2 == 0:
                        nc.vector.tensor_copy(xt[:], xt_ps[:])
                    else:
                        nc.scalar.copy(xt[:], xt_ps[:])
                    z_ps = ps_z.tile([128, SB * BL], f32)
                    nc.tensor.matmul(z_ps[:], lhsT=uw_sb[:], rhs=xt[:],
                                     start=True, stop=False)
                    nc.tensor.matmul(z_ps[:], lhsT=ww_sb[:], rhs=lmT_rep[:],
                                     start=False, stop=True)
                    tz = tzp.tile([128, SB * BL], bf16)
                    nc.scalar.activation(tz[:], z_ps[:], AF.Tanh)
                    for j in range(SB):
                        s = sb * SB + j
                        nc.tensor.matmul(
                            sc_ps[:, s:s + 1],
                            lhsT=tz[:, j * 128:(j + 1) * 128],
                            rhs=vw_sb[:, 0:1],
                            start=True, stop=True,
                        )

                # softmax over S (per row), normalization folded into alpha
                sc_sb = sm.tile([BL, S], f32)
                nc.vector.tensor_tensor(sc_sb[:], sc_ps[:], maskb_sb[:], op=ALU.add)
                neg_mx = sm.tile([BL, 1], f32)
                nc.vector.reduce_max(neg_mx[:], sc_sb[:], AX.X, negate=True)
                expsc = sm.tile([BL, S], f32)
                sum_sb = sm.tile([BL, 1], f32)
                nc.scalar.activation(expsc[:], sc_sb[:], AF.Exp,
                                     bias=neg_mx[:, 0:1], accum_out=sum_sb[:, 0:1])
                rsum = sm.tile([BL, 1], f32)
                nc.vector.reciprocal(rsum[:], sum_sb[:])
                alpha = sm.tile([BL, S], f32)
                nc.vector.tensor_scalar_mul(alpha[:], expsc[:], rsum[:, 0:1])

                # weighted memory: am *= alpha (broadcast over H), chunked over S
                CH = 20
                for c0 in range(0, S, CH):
                    nc.vector.tensor_tensor(
                        amv[:, c0:c0 + CH, :],
                        amv[:, c0:c0 + CH, :],
                        alpha[:, c0:c0 + CH].unsqueeze(2).broadcast_to([BL, CH, H]),
                        op=ALU.mult,
                    )
                # out_e^T [H, BL] = sum_s (alpha*am)_s^T  via PE transposes into PSUM
                oe_ps = ps_acc.tile([H, BL], f32, tag="oe")
                for s in range(S):
                    nc.tensor.matmul(oe_ps[:], lhsT=amv[:, s, :], rhs=ident[:],
                                     start=(s == 0), stop=(s == S - 1),
                                     is_transpose=True)
                oeT_sb = sm.tile([H, BL], f32)
                nc.vector.tensor_copy(oeT_sb[:], oe_ps[:])
                nc.sync.dma_start(out=feats_local[0:H, :], in_=oeT_sb[:])

            # ---------------- AllGather feats ----------------
            if timeline:
                gvw = gath[:].rearrange("(c f) b -> c f b", f=H2)
                for c in range(NCORES):
                    nc.sync.dma_start(out=gvw[c], in_=feats_local[:])
            else:
                nc.gpsimd.collective_compute(
                    "AllGather",
                    mybir.AluOpType.bypass,
                    replica_groups=[list(range(NCORES))],
                    ins=[feats_local[:].opt()],
                    outs=[gath[:].opt()],
                )
            gv = gath[:].rearrange("(c f) b -> c f b", f=H2)

            # ---------------- Phase 2: logits + sigmoid over local cols -------
            if not use_f32r:
                ew_top = ewp.tile([H, NL], bf16)
                nc.sync.dma_start(out=ew_top[:], in_=ew[0:H, :])
                ew_bot = ewp.tile([H, NL], bf16)
                nc.sync.dma_start(out=ew_bot[:], in_=ew[H:H2, :])
                ewl_top = ewp.tile([H, NL], bf16)
                nc.sync.dma_start(out=ewl_top[:], in_=ewl[0:H, :])
                ewl_bot = ewp.tile([H, NL], bf16)
                nc.sync.dma_start(out=ewl_bot[:], in_=ewl[H:H2, :])
            if use_f32r:
                ewr_top = ewp.tile([H, NL], f32r)
                nc.sync.dma_start(out=ewr_top[:], in_=ewf[0:H, :])
                ewr_bot = ewp.tile([H, NL], f32r)
                nc.sync.dma_start(out=ewr_bot[:], in_=ewf[H:H2, :])

            if timeline:
                out_flat = dumout[:, :]
            else:
                out_flat = out[:, :].rearrange("a b -> (a b)").unsqueeze(1)

            def scat_ap(k):
                # unique fake dep region per scatter: suppresses Tile's
                # WAW serialization between scatters (all write 0.0 at
                # host-guaranteed positions; order among them is free).
                # The real store->scatter ordering is added explicitly.
                return bass.AP(
                    tensor=out_flat.tensor, offset=0, ap=out_flat.ap,
                    dep_tracking_offset=(1 << 33) + k * (1 << 23))

            zeros_sb = None

            with (
                tc.tile_pool(name="ps2", bufs=8, space="PSUM") as ps2,
                tc.tile_pool(name="outp", bufs=2) as outp,
                tc.tile_pool(name="gp", bufs=2) as gp,
            ):
                zeros_sb = cp.tile([128, 1], f32)
                nc.vector.memset(zeros_sb[:], 0.0)
                NCHW = 512
                g_all_f = gp.tile([128, 2 * NCORES * 128], f32)
                nc.sync.dma_start(
                    out=g_all_f[:],
                    in_=gath[:].rearrange("(t p) b -> p t b", p=128))
                if not use_f32r:
                    g_all = gp.tile([128, 2 * NCORES * 128], bf16)
                    nc.vector.tensor_copy(g_all[:], g_all_f[:])
                if use_f32r:
                    g_r = gp.tile([128, 2 * NCORES * 128], f32r)
                    nc.sync.dma_start(
                        out=g_r[:],
                        in_=gath[:].rearrange("(t p) b -> p t b",
                                              p=128).bitcast(f32r))
                if not use_f32r:
                    g_rs = gp.tile([128, 2 * NCORES * 128], f32)
                    nc.vector.tensor_tensor(g_rs[:], g_all_f[:], g_all[:],
                                            op=ALU.subtract)
                    g_lo = gp.tile([128, 2 * NCORES * 128], bf16)
                    nc.vector.tensor_copy(g_lo[:], g_rs[:])
                six_all = gp.tile([128, n_scat], i32)
                nc.sync.dma_start(out=six_all[:], in_=sidx[:, :])
                scat_by_dep = {}
                for w, d in enumerate(dep_cb):
                    scat_by_dep.setdefault(d, []).append(w)
                for cb in range(NCORES):
                    if not use_f32r:
                        g_oe = g_all[:, (2 * cb) * 128:(2 * cb + 1) * 128]
                        g_lm = g_all[:, (2 * cb + 1) * 128:(2 * cb + 2) * 128]
                        gl_oe = g_lo[:, (2 * cb) * 128:(2 * cb + 1) * 128]
                        gl_lm = g_lo[:, (2 * cb + 1) * 128:(2 * cb + 2) * 128]
                    out_sb = outp.tile([128, NL], f32)
                    for n0 in range(0, NL, NCHW):
                        w = min(NCHW, NL - n0)
                        pt = ps2.tile([128, NCHW], f32)
                        for q0 in range(0, w, 512):
                            qw = min(512, w - q0)
                            sl = slice(n0 + q0, n0 + q0 + qw)
                            po = pt[:, q0:q0 + qw]
                            if use_f32r:
                                gr_oe = g_r[:, (2 * cb) * 128:(2 * cb + 1) * 128]
                                gr_lm = g_r[:, (2 * cb + 1) * 128:(2 * cb + 2) * 128]
                                nc.tensor.matmul(po, lhsT=gr_oe,
                                                 rhs=ewr_top[:, sl],
                                                 start=True, stop=False)
                                nc.tensor.matmul(po, lhsT=gr_lm,
                                                 rhs=ewr_bot[:, sl],
                                                 start=False, stop=True)
                                continue
                            if two_mm:
                                nc.tensor.matmul(po, lhsT=g_oe,
                                                 rhs=ew_top[:, sl],
                                                 start=True, stop=False)
                                nc.tensor.matmul(po, lhsT=g_lm,
                                                 rhs=ew_bot[:, sl],
                                                 start=False, stop=True)
                                continue
                            nc.tensor.matmul(po, lhsT=g_oe,
                                             rhs=ew_top[:, sl],
                                             start=True, stop=False)
                            nc.tensor.matmul(po, lhsT=g_oe,
                                             rhs=ewl_top[:, sl],
                                             start=False, stop=False)
                            nc.tensor.matmul(po, lhsT=gl_oe,
                                             rhs=ew_top[:, sl],
                                             start=False, stop=False)
                            nc.tensor.matmul(po, lhsT=g_lm,
                                             rhs=ew_bot[:, sl],
                                             start=False, stop=False)
                            nc.tensor.matmul(po, lhsT=g_lm,
                                             rhs=ewl_bot[:, sl],
                                             start=False, stop=False)
                            nc.tensor.matmul(po, lhsT=gl_lm,
                                             rhs=ew_bot[:, sl],
                                             start=False, stop=True)
                        nc.scalar.activation(out_sb[:, n0:n0 + w], pt[:, :w],
                                             AF.Sigmoid)
                    st = nc.sync.dma_start(out=out[cb * 128:(cb + 1) * 128, :],
                                           in_=out_sb[:])
                    # seen-item mask: scatter 0.0 over stored rows. HW indirect
                    # DMA consumes one offset per partition -> 128 single-
                    # element writes per instruction; instruction w carries
                    # offsets only from row-blocks <= dep_cb[w].
                    for w in ([] if no_scatter else scat_by_dep.get(cb, [])):
                        sc_inst = nc.gpsimd.indirect_dma_start(
                            out=scat_ap(w),
                            out_offset=bass.IndirectOffsetOnAxis(
                                ap=six_all[:, w:w + 1], axis=0),
                            in_=zeros_sb[:, :],
                            in_offset=None,
                            bounds_check=(127 if timeline else B * NL - 1),
                            oob_is_err=False,
                        )
                        tile.add_dep_helper(sc_inst.ins, st.ins,
                                            reason="scatter after dense store")

    nc.compile()
    return nc


def _prepare_inputs(all_memory, last_memory, item_seq, mask, U_w, W_w, V_w, E_w):
    all_memory = np.asarray(all_memory, dtype=np.float32)
    last_memory = np.asarray(last_memory, dtype=np.float32)
    item_seq = np.asarray(item_seq)
    mask = np.asarray(mask)
    import ml_dtypes
    U_w = np.ascontiguousarray(np.asarray(U_w, dtype=np.float32).astype(ml_dtypes.bfloat16))
    W_w = np.ascontiguousarray(np.asarray(W_w, dtype=np.float32).astype(ml_dtypes.bfloat16))
    V_w = np.ascontiguousarray(np.asarray(V_w, dtype=np.float32).reshape(H, 1).astype(ml_dtypes.bfloat16))
    E_w32 = np.asarray(E_w, dtype=np.float32)
    E_w = E_w32.astype(ml_dtypes.bfloat16)
    E_wlo = (E_w32 - E_w.astype(np.float32)).astype(ml_dtypes.bfloat16)

    # ----- host-side scatter index prep (per core, per 128-row block) -----
    items = item_seq.astype(np.int64)
    valid = items > 0
    core_of = items // NL
    b_idx = np.arange(B)[:, None].repeat(S, axis=1)
    flat_in_core = b_idx * NL + (items - core_of * NL)   # [B,S]

    # pack each core's offsets cb-ordered into chunks of 128 (one indirect-DMA
    # instruction each); record which store each chunk must wait for
    offs = {}
    for c in range(NCORES):
        for cb in range(NCORES):
            sel = valid & (core_of == c) & ((b_idx // 128) == cb)
            offs[(c, cb)] = flat_in_core[sel].astype(np.int32)
    totals = [sum(offs[(c, cb)].size for cb in range(NCORES))
              for c in range(NCORES)]
    n_scat = max(2, -(-max(totals) // 128))
    sidx_all = np.full((NCORES, 128, n_scat), SENTINEL, dtype=np.int32)
    dep = np.zeros((NCORES, n_scat), dtype=np.int64)
    for c in range(NCORES):
        flat = np.full(n_scat * 128, SENTINEL, dtype=np.int32)
        cbs = np.zeros(n_scat * 128, dtype=np.int64)
        pos = 0
        for cb in range(NCORES):
            o = offs[(c, cb)]
            flat[pos:pos + o.size] = o
            cbs[pos:pos + o.size] = cb
            pos += o.size
        sidx_all[c] = flat.reshape(n_scat, 128).T
        dep[c] = cbs.reshape(n_scat, 128).max(axis=1)
    dep_cb = tuple(int(x) for x in dep.max(axis=0))
    scat_w = (n_scat, dep_cb)

    maskbias = np.where(mask, np.float32(-1e9), np.float32(0.0)).astype(np.float32)
    in_maps = []
    for c in range(NCORES):
        r0, r1 = c * BL, (c + 1) * BL
        in_maps.append({
            "am": np.ascontiguousarray(
                all_memory[r0:r1].reshape(BL, S * H)),
            "lm": np.ascontiguousarray(last_memory[r0:r1]),
            "maskb": np.ascontiguousarray(maskbias[r0:r1]),
            "uw": U_w,
            "ww": W_w,
            "vw": V_w,
            "ew": np.ascontiguousarray(E_w[:, c * NL:(c + 1) * NL]),
            "ewl": np.ascontiguousarray(E_wlo[:, c * NL:(c + 1) * NL]),
            "sidx": np.ascontiguousarray(sidx_all[c]),
        })
    return scat_w, in_maps


def kernel(all_memory, last_memory, item_seq, mask, U_w, W_w, V_w, V_b, E_w):
    from concourse.bass_utils import run_bass_kernel_spmd

    scat_w, in_maps = _prepare_inputs(
        all_memory, last_memory, item_seq, mask, U_w, W_w, V_w, E_w)
    if scat_w not in _BUILT:
        _BUILT[scat_w] = _build(scat_w)
    nc = _BUILT[scat_w]
    res = run_bass_kernel_spmd(nc, in_maps, core_ids=list(range(NCORES)))
    global _LAST_RESULTS
    _LAST_RESULTS = res
    return np.concatenate([res.results[c]["out"] for c in range(NCORES)], axis=1)



# revision 20
# speedup vs baseline: 1.2590x; 1.2590x over previous
"""Trainium2 Bass kernel for nn_Explore_Decoder (scatter_memory).

Full computation:
    a      = all_memory @ U_w                         [B,S,H]
    l      = (last_memory @ W_w)[:,None,:]            [B,1,H]
    scores = (tanh(a+l) @ V_w + V_b)[...,0]           [B,S]
    scores = where(mask, -1e9, scores)
    alpha  = softmax(scores, axis=1)
    out_e  = sum(alpha * all_memory, axis=1)          [B,H]
    feats  = concat([out_e, last_memory], axis=1)     [B,2H]
    logits = feats @ E_w                              [B,N]
    logits = where(seen_item, -inf, logits)           (scatter of item_seq)
    return sigmoid(logits)

Sharding (8 cores):
  Phase 1 (attention): data-parallel over B; core c owns rows [c*128,(c+1)*128).
    Produces featsT [2H, 128] bf16 per core, AllGather -> featsT for all B.
  Phase 2 (logits): tensor-parallel over N; core c owns cols [c*6250,(c+1)*6250).
    Dense: out = sigmoid(featsT.T @ E_w[:, cols_c]) for all 1024 rows, bf16 in
    (split-free: the 2e-2 harness gate leaves plenty of room), bf16 out.
    Scatter: host-precomputed flat offsets of seen items in the local column
    range; one wide indirect DMA per 128-row block writes 0.0 ( = sigmoid(-inf))
    over the dense output.
"""

import numpy as np

B, S, H, N = 1024, 100, 128, 50000
NCORES = 8
BL = B // NCORES          # 128 batch rows per core (phase 1)
NL = N // NCORES          # 6250 vocab cols per core (phase 2)
H2 = 2 * H
SB = 4                    # s-values per phase-1 block
NSB = S // SB             # 25 blocks
SENTINEL = np.int32(2**30)

_BUILT = {}               # scat_w -> compiled Bass module
_LAST_RESULTS = None      # BassKernelResults of the most recent run (for tests)


def _default_spec(n_scat=100):
    # representative scatter spec for profiling: uniform item distribution
    per = 12800 / NCORES
    dep = tuple(min(NCORES - 1, int((w + 1) * 128 / per)) for w in range(n_scat))
    return (n_scat, dep)


def _build(scat_w, reps: int = 1, timeline: bool = False,
           no_scatter: bool = False):
    # scat_w: K = offset columns per 128-row output block (from host packing)
    import concourse.bass as bass
    import concourse.mybir as mybir
    import concourse.tile as tile
    from concourse import bacc
    from concourse.masks import make_identity

    f32 = mybir.dt.float32
    bf16 = mybir.dt.bfloat16
    i32 = mybir.dt.int32
    AF = mybir.ActivationFunctionType
    ALU = mybir.AluOpType
    AX = mybir.AxisListType

    n_scat, dep_cb = scat_w

    nc = bacc.Bacc(None, target_bir_lowering=False, debug=False)

    am = nc.dram_tensor("am", [BL, S * H], bf16, kind="ExternalInput")
    # lmk = [last_memory | maskmul] rows; wcat = [U_w | W_w | V_w] columns
    lmk = nc.dram_tensor("lmk", [BL, H + S], f32, kind="ExternalInput")
    wcat = nc.dram_tensor("wcat", [H, H + H + 1], bf16, kind="ExternalInput")
    ew = nc.dram_tensor("ew", [H2, NL], bf16, kind="ExternalInput")
    # [p, w] = flat element offset (into the [B, NL] output) for partition p of
    # indirect-DMA instruction w; instruction w may fire once the store of
    # row-block dep_cb[w] has landed. SENTINEL = unused slot.
    sidx = nc.dram_tensor("sidx", [128, n_scat], i32, kind="ExternalInput")
    # 9 virtual row-blocks: slot 0 = this core's own rows (computed from local
    # SBUF feats BEFORE the AllGather so its store + scatters start early);
    # slots 1..8 = gathered blocks 0..7 (slot 1+c duplicates slot 0 unmasked;
    # the host drops it at unshard).
    out = nc.dram_tensor("out", [B + BL, NL], bf16, kind="ExternalOutput")
    # timeline mode: single-core cost-model sim can't price collectives or
    # full-tensor indirect APs; swap in traffic-equivalent stand-ins
    dumout = nc.dram_tensor("dumout", [128, 1], bf16) if timeline else None

    with tile.TileContext(nc) as tc:
      for _rep in range(reps):
        with (
            tc.tile_pool(name="consts", bufs=1) as cp,
            tc.tile_pool(name="amp", bufs=1) as amp,
            tc.tile_pool(name="ewp", bufs=1) as ewp,
            tc.tile_pool(name="dram", bufs=1, space="DRAM") as dp,
            tc.tile_pool(name="smax", bufs=1) as sm,
        ):
            # two batched const DMAs first (lm unblocks the in-order PE
            # queue's first transpose), then the am stream, then ew, then sidx
            ident = cp.tile([128, 128], f32)
            make_identity(nc, ident[:])
            ident_bf = cp.tile([128, 128], bf16)
            nc.vector.tensor_copy(ident_bf[:], ident[:])
            lmk_sb = cp.tile([BL, H + S], f32)
            nc.sync.dma_start(out=lmk_sb[:], in_=lmk[:, :])
            lm_sb = lmk_sb[:, 0:H]
            maskb_sb = lmk_sb[:, H:H + S]
            wcat_sb = cp.tile([H, H + H + 1], bf16)
            nc.sync.dma_start(out=wcat_sb[:], in_=wcat[:, :])
            uw_sb = wcat_sb[:, 0:H]
            ww_sb = wcat_sb[:, H:2 * H]
            vw_sb = wcat_sb[:, 2 * H:2 * H + 1]
            zeros_sb = cp.tile([128, 1], bf16)
            nc.vector.memset(zeros_sb[:], 0.0)

            am_t = amp.tile([BL, S * H], bf16)
            AMCH = 20 * H
            for a0 in range(0, S * H, AMCH):
                nc.sync.dma_start(out=am_t[:, a0:a0 + AMCH],
                                  in_=am[:, a0:a0 + AMCH])
            amv = am_t[:].rearrange("p (s h) -> p s h", h=H)

            # phase-2 weights: no deps, needed only ~20us in
            ew_top = ewp.tile([H, NL], bf16)
            nc.sync.dma_start(out=ew_top[:], in_=ew[0:H, :])
            ew_bot = ewp.tile([H, NL], bf16)
            nc.sync.dma_start(out=ew_bot[:], in_=ew[H:H2, :])
            six_all = cp.tile([128, n_scat], i32)
            nc.sync.dma_start(out=six_all[:], in_=sidx[:, :])

            feats_local = dp.tile([H2, BL], bf16)
            gath = dp.tile([NCORES * H2, BL], bf16)

            # ---------------- Phase 1: attention over S, rows of this core ----
            with (
                tc.tile_pool(name="ps_t", bufs=3, space="PSUM") as ps_t,
                tc.tile_pool(name="ps_z", bufs=2, space="PSUM") as ps_z,
                tc.tile_pool(name="ps_acc", bufs=1, space="PSUM") as ps_acc,
                tc.tile_pool(name="xtp", bufs=6) as xtp,
                tc.tile_pool(name="tzp", bufs=6) as tzp,
            ):
                # last_memory^T  [H, BL] , replicated x SB for the Z matmul rhs
                lmT_ps = ps_t.tile([128, 512], f32, tag="tps")
                nc.tensor.transpose(out=lmT_ps[:, :H], in_=lm_sb,
                                    identity=ident[:])
                lmT_sb = cp.tile([H, BL], f32)
                nc.vector.tensor_copy(lmT_sb[:], lmT_ps[:, :H])
                lmT_rep = cp.tile([H, SB * BL], bf16)
                nc.vector.tensor_copy(
                    lmT_rep[:].rearrange("h (s b) -> h s b", s=SB),
                    lmT_sb[:].unsqueeze(1).broadcast_to([H, SB, BL]),
                )
                # feats rows H..2H = last_memory^T (raw), bf16
                lmT_bf = cp.tile([H, BL], bf16)
                nc.vector.tensor_copy(lmT_bf[:], lmT_sb[:])
                nc.sync.dma_start(out=feats_local[H:H2, :], in_=lmT_bf[:])

                # Online (fused) attention: scores are tanh-bounded
                # (|sc| <= ||V||_1 ~ 2.6), so exp without max-subtraction is
                # safe; the mask is a 0/1 multiplier on exp(sc); the
                # alpha-weighted readout accumulates per block inside the
                # loop; 1/sum folds into one diag matmul at the end.
                sc_ps = ps_acc.tile([BL, S], f32, tag="sc")
                oe_ps = ps_acc.tile([H, BL], f32, tag="oe")
                alpham = sm.tile([BL, S], f32)   # masked exp(sc), unnormalized
                with tc.tile_pool(name="wamp", bufs=3) as wamp:
                  for sb in range(NSB):
                    sl = slice(sb * SB, (sb + 1) * SB)
                    xt_ps = ps_t.tile([128, SB * 128], bf16, tag="tps")
                    for j in range(SB):
                        s = sb * SB + j
                        nc.tensor.transpose(
                            out=xt_ps[:, j * 128:(j + 1) * 128],
                            in_=amv[:, s, :],
                            identity=ident_bf[:],
                        )
                    xt = xtp.tile([128, SB * 128], bf16)
                    if sb % 2 == 0:
                        nc.vector.tensor_copy(xt[:], xt_ps[:])
                    else:
                        nc.scalar.copy(xt[:], xt_ps[:])
                    z_ps = ps_z.tile([128, SB * BL], f32)
                    nc.tensor.matmul(z_ps[:], lhsT=uw_sb, rhs=xt[:],
                                     start=True, stop=False)
                    nc.tensor.matmul(z_ps[:], lhsT=ww_sb, rhs=lmT_rep[:],
                                     start=False, stop=True)
                    tz = tzp.tile([128, SB * BL], bf16)
                    nc.scalar.activation(tz[:], z_ps[:], AF.Tanh)
                    for j in range(SB):
                        s = sb * SB + j
                        nc.tensor.matmul(
                            sc_ps[:, s:s + 1],
                            lhsT=tz[:, j * 128:(j + 1) * 128],
                            rhs=vw_sb,
                            start=True, stop=True,
                        )
                    # unnormalized masked weights for this block
                    nc.scalar.activation(alpham[:, sl], sc_ps[:, sl], AF.Exp)
                    nc.gpsimd.tensor_tensor(alpham[:, sl], alpham[:, sl],
                                            maskb_sb[:, sl], op=ALU.mult)
                    wam = wamp.tile([BL, SB * H], f32)
                    wv = wam[:].rearrange("p (s h) -> p s h", h=H)
                    nc.vector.tensor_tensor(
                        wv,
                        amv[:, sl, :],
                        alpham[:, sl].unsqueeze(2).broadcast_to([BL, SB, H]),
                        op=ALU.mult,
                    )
                    for j in range(SB):
                        s = sb * SB + j
                        nc.tensor.matmul(oe_ps[:], lhsT=wv[:, j, :],
                                         rhs=ident[:],
                                         start=(s == 0), stop=(s == S - 1),
                                         is_transpose=True)

                # normalization: oeT = O^T @ diag(1/sum)
                sum_sb = sm.tile([BL, 1], f32)
                nc.vector.tensor_reduce(sum_sb[:], alpham[:], AX.X, op=ALU.add)
                rsum = sm.tile([BL, 1], f32)
                nc.vector.reciprocal(rsum[:], sum_sb[:])
                # preload the Sigmoid act table (keeps the table switch off
                # slot-0's critical path)
                sigwarm = sm.tile([BL, 1], f32)
                nc.scalar.activation(sigwarm[:], rsum[:], AF.Sigmoid)
                rdiag = sm.tile([BL, BL], bf16)
                nc.vector.tensor_scalar_mul(rdiag[:], ident_bf[:],
                                            rsum[:, 0:1])
                OT_sb = sm.tile([H, BL], f32)
                nc.vector.tensor_copy(OT_sb[:], oe_ps[:])
                O_ps = ps_t.tile([128, 512], f32, tag="tps")
                nc.tensor.transpose(out=O_ps[:, :H], in_=OT_sb[:],
                                    identity=ident[:])
                O_sb = sm.tile([BL, H], bf16)
                nc.vector.tensor_copy(O_sb[:], O_ps[:, :H])
                oeT_ps = ps_acc.tile([H, BL], f32, tag="sc")
                nc.tensor.matmul(oeT_ps[:], lhsT=O_sb[:], rhs=rdiag[:],
                                 start=True, stop=True)
                oeT_sb = sm.tile([H, BL], bf16)
                nc.vector.tensor_copy(oeT_sb[:], oeT_ps[:])
                nc.sync.dma_start(out=feats_local[0:H, :], in_=oeT_sb[:])

            # ---------------- AllGather feats ----------------
            if timeline:
                gvw = gath[:].rearrange("(c f) b -> c f b", f=H2)
                nc.sync.dma_start(
                    out=gvw[:, :, :],
                    in_=feats_local[:].unsqueeze(0).broadcast_to(
                        [NCORES, H2, BL]))
            else:
                nc.gpsimd.collective_compute(
                    "AllGather",
                    mybir.AluOpType.bypass,
                    replica_groups=[list(range(NCORES))],
                    ins=[feats_local[:].opt()],
                    outs=[gath[:].opt()],
                )

            # ---------------- Phase 2: logits + sigmoid over local cols -------
            if timeline:
                out_flat = dumout[:, :].rearrange("a b -> (a b)").unsqueeze(1)
            else:
                out_flat = out[:, :].rearrange("a b -> (a b)").unsqueeze(1)

            def scat_ap(k):
                # unique fake dep region per scatter: suppresses Tile's
                # WAW serialization between scatters (all write 0.0 at
                # host-guaranteed positions; order among them is free).
                # The real store->scatter ordering is added explicitly.
                return bass.AP(
                    tensor=out_flat.tensor, offset=0, ap=out_flat.ap,
                    dep_tracking_offset=(1 << 33) + k * (1 << 23))

            with (
                tc.tile_pool(name="ps2", bufs=4, space="PSUM") as ps2,
                tc.tile_pool(name="outp", bufs=2) as outp,
                tc.tile_pool(name="gp", bufs=1) as gp,
            ):
                NCHW = 1024
                # featsT for all cores: [h or h+128, (core, b)] as [128, 2048]
                # (load emitted after slot-0's store so it doesn't delay it
                # on the DMA engines)
                g_all = gp.tile([128, 2 * NCORES * 128], bf16)
                scat_by_dep = {}
                for w, d in enumerate(dep_cb):
                    scat_by_dep.setdefault(d, []).append(w)
                for cb in range(NCORES + 1):
                    if cb == 0:
                        g_oe = oeT_sb[:]
                        g_lm = lmT_bf[:]
                    elif cb == 1:
                        nc.sync.dma_start(
                            out=g_all[:],
                            in_=gath[:].rearrange("(t p) b -> p t b", p=128))
                        p = cb - 1
                        g_oe = g_all[:, (2 * p) * 128:(2 * p + 1) * 128]
                        g_lm = g_all[:, (2 * p + 1) * 128:(2 * p + 2) * 128]
                    else:
                        p = cb - 1
                        g_oe = g_all[:, (2 * p) * 128:(2 * p + 1) * 128]
                        g_lm = g_all[:, (2 * p + 1) * 128:(2 * p + 2) * 128]
                    out_sb = outp.tile([128, NL], bf16)
                    for n0 in range(0, NL, NCHW):
                        w = min(NCHW, NL - n0)
                        pt = ps2.tile([128, NCHW], f32)
                        for q0 in range(0, w, 512):
                            qw = min(512, w - q0)
                            sl = slice(n0 + q0, n0 + q0 + qw)
                            po = pt[:, q0:q0 + qw]
                            nc.tensor.matmul(po, lhsT=g_oe,
                                             rhs=ew_top[:, sl],
                                             start=True, stop=False)
                            nc.tensor.matmul(po, lhsT=g_lm,
                                             rhs=ew_bot[:, sl],
                                             start=False, stop=True)
                        nc.scalar.activation(out_sb[:, n0:n0 + w], pt[:, :w],
                                             AF.Sigmoid)
                    st = nc.sync.dma_start(out=out[cb * 128:(cb + 1) * 128, :],
                                           in_=out_sb[:])
                    # seen-item mask: scatter 0.0 over stored rows. HW indirect
                    # DMA consumes one offset per partition -> 128 single-
                    # element writes per instruction; instruction w carries
                    # offsets only from row-blocks <= dep_cb[w].
                    for w in ([] if no_scatter else scat_by_dep.get(cb, [])):
                        sc_inst = nc.gpsimd.indirect_dma_start(
                            out=scat_ap(w),
                            out_offset=bass.IndirectOffsetOnAxis(
                                ap=six_all[:, w:w + 1], axis=0),
                            in_=zeros_sb[:, :],
                            in_offset=None,
                            bounds_check=(127 if timeline else (B + BL) * NL - 1),
                            oob_is_err=False,
                        )
                        tile.add_dep_helper(sc_inst.ins, st.ins,
                                            reason="scatter after dense store")

    nc.compile()
    return nc


def _prepare_inputs(all_memory, last_memory, item_seq, mask, U_w, W_w, V_w, E_w):
    import ml_dtypes
    bf = ml_dtypes.bfloat16
    all_memory = np.asarray(all_memory, dtype=np.float32).astype(bf)
    last_memory = np.asarray(last_memory, dtype=np.float32)
    item_seq = np.asarray(item_seq)
    mask = np.asarray(mask)
    U_w = np.ascontiguousarray(np.asarray(U_w, dtype=np.float32).astype(bf))
    W_w = np.ascontiguousarray(np.asarray(W_w, dtype=np.float32).astype(bf))
    V_w = np.ascontiguousarray(
        np.asarray(V_w, dtype=np.float32).reshape(H, 1).astype(bf))
    E_w = np.asarray(E_w, dtype=np.float32).astype(bf)

    # ----- host-side scatter index prep (per core, per virtual slot) -----
    # core c's out tensor has 9 row-blocks: slot 0 = physical block c (own
    # rows, stored first), slot 1+p = physical block p. Element (b, col) of
    # core c maps to row9 = b%128 if b//128==c else (1+b//128)*128 + b%128.
    items = item_seq.astype(np.int64)
    valid = items > 0
    core_of = items // NL
    b_idx = np.arange(B)[:, None].repeat(S, axis=1)
    col_in_core = items - core_of * NL                   # [B,S]

    offs = {}
    for c in range(NCORES):
        for slot in range(NCORES + 1):
            if slot == 1 + c:
                offs[(c, slot)] = np.zeros(0, np.int32)
                continue
            p = c if slot == 0 else slot - 1
            sel = valid & (core_of == c) & ((b_idx // 128) == p)
            row9 = slot * 128 + (b_idx[sel] % 128)
            offs[(c, slot)] = (row9 * NL + col_in_core[sel]).astype(np.int32)
    totals = [sum(offs[(c, s)].size for s in range(NCORES + 1))
              for c in range(NCORES)]
    n_scat = max(2, -(-max(totals) // 128))
    sidx_all = np.full((NCORES, 128, n_scat), SENTINEL, dtype=np.int32)
    dep = np.zeros((NCORES, n_scat), dtype=np.int64)
    for c in range(NCORES):
        flat = np.full(n_scat * 128, SENTINEL, dtype=np.int32)
        cbs = np.zeros(n_scat * 128, dtype=np.int64)
        pos = 0
        for slot in range(NCORES + 1):
            o = offs[(c, slot)]
            flat[pos:pos + o.size] = o
            cbs[pos:pos + o.size] = slot
            pos += o.size
        sidx_all[c] = flat.reshape(n_scat, 128).T
        dep[c] = cbs.reshape(n_scat, 128).max(axis=1)
    dep_cb = tuple(int(x) for x in dep.max(axis=0))
    scat_w = (n_scat, dep_cb)

    # multiplicative mask on exp(scores): 0 at masked positions
    maskbias = np.where(mask, np.float32(0.0), np.float32(1.0)).astype(np.float32)
    in_maps = []
    for c in range(NCORES):
        r0, r1 = c * BL, (c + 1) * BL
        in_maps.append({
            "am": np.ascontiguousarray(
                all_memory[r0:r1].reshape(BL, S * H)),
            "lm": np.ascontiguousarray(last_memory[r0:r1]),
            "maskb": np.ascontiguousarray(maskbias[r0:r1]),
            "uw": U_w,
            "ww": W_w,
            "vw": V_w,
            "ew": np.ascontiguousarray(E_w[:, c * NL:(c + 1) * NL]),
            "sidx": np.ascontiguousarray(sidx_all[c]),
        })
    return scat_w, in_maps


def kernel(all_memory, last_memory, item_seq, mask, U_w, W_w, V_w, V_b, E_w):
    from concourse.bass_utils import run_bass_kernel_spmd

    scat_w, in_maps = _prepare_inputs(
        all_memory, last_memory, item_seq, mask, U_w, W_w, V_w, E_w)
    if scat_w not in _BUILT:
        _BUILT[scat_w] = _build(scat_w)
    nc = _BUILT[scat_w]
    res = run_bass_kernel_spmd(nc, in_maps, core_ids=list(range(NCORES)))
    global _LAST_RESULTS
    _LAST_RESULTS = res
    shards = []
    for c in range(NCORES):
        o9 = res.results[c]["out"]          # [B + BL, NL], 9 virtual blocks
        shard = o9[BL:].copy()              # physical blocks 0..7 (slots 1..8)
        shard[c * BL:(c + 1) * BL] = o9[0:BL]   # own block: masked copy
        shards.append(shard)
    return np.concatenate(shards, axis=1).astype(np.float32)


# revision 25
# speedup vs baseline: 1.4384x; 1.1425x over previous
"""Trainium2 Bass kernel for nn_Explore_Decoder (scatter_memory).

Full computation:
    a      = all_memory @ U_w                         [B,S,H]
    l      = (last_memory @ W_w)[:,None,:]            [B,1,H]
    scores = (tanh(a+l) @ V_w + V_b)[...,0]           [B,S]
    scores = where(mask, -1e9, scores)
    alpha  = softmax(scores, axis=1)
    out_e  = sum(alpha * all_memory, axis=1)          [B,H]
    feats  = concat([out_e, last_memory], axis=1)     [B,2H]
    logits = feats @ E_w                              [B,N]
    logits = where(seen_item, -inf, logits)           (scatter of item_seq)
    return sigmoid(logits)

Sharding (8 cores):
  Phase 1 (attention): data-parallel over B; core c owns rows [c*128,(c+1)*128).
    Produces featsT [2H, 128] bf16 per core, AllGather -> featsT for all B.
  Phase 2 (logits): tensor-parallel over N; core c owns cols [c*6250,(c+1)*6250).
    Dense: out = sigmoid(featsT.T @ E_w[:, cols_c]) for all 1024 rows, bf16 in
    (split-free: the 2e-2 harness gate leaves plenty of room), bf16 out.
    Scatter: host-precomputed flat offsets of seen items in the local column
    range; one wide indirect DMA per 128-row block writes 0.0 ( = sigmoid(-inf))
    over the dense output.
"""

import numpy as np

B, S, H, N = 1024, 100, 128, 50000
NCORES = 8
BL = B // NCORES          # 128 batch rows per core (phase 1)
NL = N // NCORES          # 6250 vocab cols per core (phase 2)
H2 = 2 * H
SB = 4                    # s-values per phase-1 block
SENTINEL = np.int32(2**30)

_BUILT = {}               # scat_w -> compiled Bass module
_LAST_RESULTS = None      # BassKernelResults of the most recent run (for tests)


def _default_spec(n_scat=100):
    # representative scatter spec for profiling: uniform item distribution
    per = 12800 / NCORES
    dep = tuple(min(NCORES - 1, int((w + 1) * 128 / per)) for w in range(n_scat))
    return (n_scat, dep)


def _build(scat_w, reps: int = 1, timeline: bool = False,
           no_scatter: bool = False):
    # scat_w: K = offset columns per 128-row output block (from host packing)
    import concourse.bass as bass
    import concourse.mybir as mybir
    import concourse.tile as tile
    from concourse import bacc
    from concourse.masks import make_identity

    f32 = mybir.dt.float32
    bf16 = mybir.dt.bfloat16
    i32 = mybir.dt.int32
    AF = mybir.ActivationFunctionType
    ALU = mybir.AluOpType
    AX = mybir.AxisListType

    n_scat, dep_cb = scat_w

    nc = bacc.Bacc(None, target_bir_lowering=False, debug=False)

    am = nc.dram_tensor("am", [BL, S * H], bf16, kind="ExternalInput")
    # lmk = [last_memory | maskmul] rows; wcat = [U_w | W_w | V_w] columns
    lmk = nc.dram_tensor("lmk", [BL, H + S], f32, kind="ExternalInput")
    wcat = nc.dram_tensor("wcat", [H, H + H + 1], bf16, kind="ExternalInput")
    ew = nc.dram_tensor("ew", [H2, NL], bf16, kind="ExternalInput")
    # [p, w] = flat element offset (into the [B, NL] output) for partition p of
    # indirect-DMA instruction w; instruction w may fire once the store of
    # row-block dep_cb[w] has landed. SENTINEL = unused slot.
    sidx = nc.dram_tensor("sidx", [128, n_scat], i32, kind="ExternalInput")
    # 9 virtual row-blocks: slot 0 = this core's own rows (computed from local
    # SBUF feats BEFORE the AllGather so its store + scatters start early);
    # slots 1..8 = gathered blocks 0..7 (slot 1+c duplicates slot 0 unmasked;
    # the host drops it at unshard).
    out = nc.dram_tensor("out", [B + BL, NL], bf16, kind="ExternalOutput")
    # timeline mode: single-core cost-model sim can't price collectives or
    # full-tensor indirect APs; swap in traffic-equivalent stand-ins
    dumout = nc.dram_tensor("dumout", [128, 1], bf16) if timeline else None

    with tile.TileContext(nc) as tc:
      for _rep in range(reps):
        with (
            tc.tile_pool(name="consts", bufs=1) as cp,
            tc.tile_pool(name="amp", bufs=1) as amp,
            tc.tile_pool(name="ewp", bufs=1) as ewp,
            tc.tile_pool(name="dram", bufs=1, space="DRAM") as dp,
            tc.tile_pool(name="smax", bufs=1) as sm,
        ):
            # two batched const DMAs first (lm unblocks the in-order PE
            # queue's first transpose), then the am stream, then ew, then sidx
            ident = cp.tile([128, 128], f32)
            make_identity(nc, ident[:])
            ident_bf = cp.tile([128, 128], bf16)
            nc.vector.tensor_copy(ident_bf[:], ident[:])
            lmk_sb = cp.tile([BL, H + S], f32)
            nc.sync.dma_start(out=lmk_sb[:], in_=lmk[:, :])
            lm_sb = lmk_sb[:, 0:H]
            maskb_sb = lmk_sb[:, H:H + S]
            wcat_sb = cp.tile([H, H + H + 1], bf16)
            nc.sync.dma_start(out=wcat_sb[:], in_=wcat[:, :])
            uw_sb = wcat_sb[:, 0:H]
            ww_sb = wcat_sb[:, H:2 * H]
            vw_sb = wcat_sb[:, 2 * H:2 * H + 1]
            zeros_sb = cp.tile([128, 1], bf16)
            nc.vector.memset(zeros_sb[:], 0.0)

            am_t = amp.tile([BL, S * H], bf16)
            AMCH = 20 * H
            for a0 in range(0, S * H, AMCH):
                nc.sync.dma_start(out=am_t[:, a0:a0 + AMCH],
                                  in_=am[:, a0:a0 + AMCH])
            amv = am_t[:].rearrange("p (s h) -> p s h", h=H)

            # phase-2 weights: no deps, needed only ~20us in
            ew_top = ewp.tile([H, NL], bf16)
            nc.sync.dma_start(out=ew_top[:], in_=ew[0:H, :])
            ew_bot = ewp.tile([H, NL], bf16)
            nc.sync.dma_start(out=ew_bot[:], in_=ew[H:H2, :])
            six_all = cp.tile([128, n_scat], i32)
            nc.sync.dma_start(out=six_all[:], in_=sidx[:, :])

            feats_local = dp.tile([H2, BL], bf16)
            gath = dp.tile([NCORES * H2, BL], bf16)

            # ---------------- Phase 1: attention over S, rows of this core ----
            with (
                tc.tile_pool(name="ps_t", bufs=3, space="PSUM") as ps_t,
                tc.tile_pool(name="ps_z", bufs=2, space="PSUM") as ps_z,
                tc.tile_pool(name="ps_acc", bufs=1, space="PSUM") as ps_acc,
                tc.tile_pool(name="xtp", bufs=6) as xtp,
                tc.tile_pool(name="tzp", bufs=6) as tzp,
            ):
                # last_memory^T  [H, BL] , replicated x SB for the Z matmul rhs
                lmT_ps = ps_t.tile([128, 512], f32, tag="tps")
                nc.tensor.transpose(out=lmT_ps[:, :H], in_=lm_sb,
                                    identity=ident[:])
                lmT_sb = cp.tile([H, BL], f32)
                nc.vector.tensor_copy(lmT_sb[:], lmT_ps[:, :H])
                lmT_rep = cp.tile([H, SB * BL], bf16)
                nc.vector.tensor_copy(
                    lmT_rep[:].rearrange("h (s b) -> h s b", s=SB),
                    lmT_sb[:].unsqueeze(1).broadcast_to([H, SB, BL]),
                )
                # feats rows H..2H = last_memory^T (raw), bf16
                lmT_bf = cp.tile([H, BL], bf16)
                nc.vector.tensor_copy(lmT_bf[:], lmT_sb[:])
                nc.sync.dma_start(out=feats_local[H:H2, :], in_=lmT_bf[:])

                # Online (fused) attention: scores are tanh-bounded
                # (|sc| <= ||V||_1 ~ 2.6), so exp without max-subtraction is
                # safe; the mask is a 0/1 multiplier on exp(sc); the
                # alpha-weighted readout accumulates per block inside the
                # loop; 1/sum folds into one diag matmul at the end.
                sc_ps = ps_acc.tile([BL, S], f32, tag="sc")
                oe_ps = ps_acc.tile([H, BL], f32, tag="oe")
                alpham = sm.tile([BL, S], f32)   # masked exp(sc), unnormalized
                with tc.tile_pool(name="wamp", bufs=3) as wamp:
                  for bi, s0 in enumerate(range(0, S, SB)):
                    sw = min(SB, S - s0)
                    sl = slice(s0, s0 + sw)
                    xt_ps = ps_t.tile([128, SB * 128], bf16, tag="tps")
                    for j in range(sw):
                        nc.tensor.transpose(
                            out=xt_ps[:, j * 128:(j + 1) * 128],
                            in_=amv[:, s0 + j, :],
                            identity=ident_bf[:],
                        )
                    xt = xtp.tile([128, SB * 128], bf16)
                    if bi % 2 == 0:
                        nc.vector.tensor_copy(xt[:, :sw * 128],
                                              xt_ps[:, :sw * 128])
                    else:
                        nc.scalar.copy(xt[:, :sw * 128], xt_ps[:, :sw * 128])
                    z_ps = ps_z.tile([128, SB * BL], f32)
                    nc.tensor.matmul(z_ps[:, :sw * BL], lhsT=uw_sb,
                                     rhs=xt[:, :sw * 128],
                                     start=True, stop=False)
                    nc.tensor.matmul(z_ps[:, :sw * BL], lhsT=ww_sb,
                                     rhs=lmT_rep[:, :sw * BL],
                                     start=False, stop=True)
                    tz = tzp.tile([128, SB * BL], bf16)
                    nc.scalar.activation(tz[:, :sw * BL], z_ps[:, :sw * BL],
                                         AF.Tanh)
                    for j in range(sw):
                        s = s0 + j
                        nc.tensor.matmul(
                            sc_ps[:, s:s + 1],
                            lhsT=tz[:, j * 128:(j + 1) * 128],
                            rhs=vw_sb,
                            start=True, stop=True,
                        )
                    # unnormalized masked weights for this block
                    nc.scalar.activation(alpham[:, sl], sc_ps[:, sl], AF.Exp)
                    nc.vector.tensor_tensor(alpham[:, sl], alpham[:, sl],
                                            maskb_sb[:, sl], op=ALU.mult)
                    wam = wamp.tile([BL, SB * H], f32)
                    wv = wam[:].rearrange("p (s h) -> p s h", h=H)
                    nc.vector.tensor_tensor(
                        wv[:, :sw, :],
                        amv[:, sl, :],
                        alpham[:, sl].unsqueeze(2).broadcast_to([BL, sw, H]),
                        op=ALU.mult,
                    )
                    for j in range(sw):
                        s = s0 + j
                        nc.tensor.matmul(oe_ps[:], lhsT=wv[:, j, :],
                                         rhs=ident[:],
                                         start=(s == 0), stop=(s == S - 1),
                                         is_transpose=True)

                # normalization: oeT = O^T @ diag(1/sum)
                sum_sb = sm.tile([BL, 1], f32)
                nc.vector.tensor_reduce(sum_sb[:], alpham[:], AX.X, op=ALU.add)
                rsum = sm.tile([BL, 1], f32)
                nc.vector.reciprocal(rsum[:], sum_sb[:])
                # preload the Sigmoid act table (keeps the table switch off
                # slot-0's critical path)
                sigwarm = sm.tile([BL, 1], f32)
                nc.scalar.activation(sigwarm[:], rsum[:], AF.Sigmoid)
                rdiag = sm.tile([BL, BL], bf16)
                nc.vector.tensor_scalar_mul(rdiag[:], ident_bf[:],
                                            rsum[:, 0:1])
                OT_sb = sm.tile([H, BL], f32)
                nc.vector.tensor_copy(OT_sb[:], oe_ps[:])
                O_ps = ps_t.tile([128, 512], f32, tag="tps")
                nc.tensor.transpose(out=O_ps[:, :H], in_=OT_sb[:],
                                    identity=ident[:])
                O_sb = sm.tile([BL, H], bf16)
                nc.vector.tensor_copy(O_sb[:], O_ps[:, :H])
                oeT_ps = ps_acc.tile([H, BL], f32, tag="sc")
                nc.tensor.matmul(oeT_ps[:], lhsT=O_sb[:], rhs=rdiag[:],
                                 start=True, stop=True)
                oeT_sb = sm.tile([H, BL], bf16)
                nc.vector.tensor_copy(oeT_sb[:], oeT_ps[:])
                nc.sync.dma_start(out=feats_local[0:H, :], in_=oeT_sb[:])

            # ---------------- AllGather feats ----------------
            if timeline:
                gvw = gath[:].rearrange("(c f) b -> c f b", f=H2)
                nc.sync.dma_start(
                    out=gvw[:, :, :],
                    in_=feats_local[:].unsqueeze(0).broadcast_to(
                        [NCORES, H2, BL]))
            else:
                nc.gpsimd.collective_compute(
                    "AllGather",
                    mybir.AluOpType.bypass,
                    replica_groups=[list(range(NCORES))],
                    ins=[feats_local[:].opt()],
                    outs=[gath[:].opt()],
                )

            # ---------------- Phase 2: logits + sigmoid over local cols -------
            if timeline:
                out_flat = dumout[:, :].rearrange("a b -> (a b)").unsqueeze(1)
            else:
                out_flat = out[:, :].rearrange("a b -> (a b)").unsqueeze(1)

            def scat_ap(k):
                # unique fake dep region per scatter: suppresses Tile's
                # WAW serialization between scatters (all write 0.0 at
                # host-guaranteed positions; order among them is free).
                # The real store->scatter ordering is added explicitly.
                return bass.AP(
                    tensor=out_flat.tensor, offset=0, ap=out_flat.ap,
                    dep_tracking_offset=(1 << 33) + k * (1 << 23))

            with (
                tc.tile_pool(name="ps2", bufs=3, space="PSUM") as ps2,
                tc.tile_pool(name="pswarm", bufs=1, space="PSUM") as pswarm,
                tc.tile_pool(name="outp", bufs=2) as outp,
                tc.tile_pool(name="gp", bufs=3) as gp,
            ):
                NCHW = 1024
                scat_by_dep = {}
                for w, d in enumerate(dep_cb):
                    scat_by_dep.setdefault(d, []).append(w)
                for cb in range(NCORES + 1):
                    if cb == 0:
                        g_oe = oeT_sb[:]
                        g_lm = lmT_bf[:]
                    else:
                        if cb == 1:
                            # slots 1..8 stall on the AllGather; keep the PE
                            # p-state ramp hot with filler transposes
                            warm_ps = pswarm.tile([128, 128], f32)
                            for _ in range(40):
                                nc.tensor.transpose(out=warm_ps[:],
                                                    in_=ident[:],
                                                    identity=ident[:])
                        p = cb - 1
                        # per-slot feats load: [h, (oe|lm), b] as [128, 256]
                        g_slot = gp.tile([128, 2 * 128], bf16)
                        nc.sync.dma_start(
                            out=g_slot[:],
                            in_=gath[p * H2:(p + 1) * H2, :].rearrange(
                                "(t p2) b -> p2 t b", p2=128))
                        g_oe = g_slot[:, 0:128]
                        g_lm = g_slot[:, 128:256]
                    out_sb = outp.tile([128, NL], bf16)
                    sts = []
                    for n0 in range(0, NL, NCHW):
                        w = min(NCHW, NL - n0)
                        pt = ps2.tile([128, NCHW], f32)
                        for q0 in range(0, w, 512):
                            qw = min(512, w - q0)
                            sl = slice(n0 + q0, n0 + q0 + qw)
                            po = pt[:, q0:q0 + qw]
                            nc.tensor.matmul(po, lhsT=g_oe,
                                             rhs=ew_top[:, sl],
                                             start=True, stop=False)
                            nc.tensor.matmul(po, lhsT=g_lm,
                                             rhs=ew_bot[:, sl],
                                             start=False, stop=True)
                        nc.scalar.activation(out_sb[:, n0:n0 + w], pt[:, :w],
                                             AF.Sigmoid)
                        # store per chunk: the slot's last store (and thus its
                        # scatters) completes ~1 chunk after the last sigmoid
                        sts.append(nc.sync.dma_start(
                            out=out[cb * 128:(cb + 1) * 128, n0:n0 + w],
                            in_=out_sb[:, n0:n0 + w]))
                    # seen-item mask: scatter 0.0 over stored rows. HW indirect
                    # DMA consumes one offset per partition -> 128 single-
                    # element writes per instruction; instruction w carries
                    # offsets only from row-blocks <= dep_cb[w].
                    for w in ([] if no_scatter else scat_by_dep.get(cb, [])):
                        sc_inst = nc.gpsimd.indirect_dma_start(
                            out=scat_ap(w),
                            out_offset=bass.IndirectOffsetOnAxis(
                                ap=six_all[:, w:w + 1], axis=0),
                            in_=zeros_sb[:, :],
                            in_offset=None,
                            bounds_check=(127 if timeline else (B + BL) * NL - 1),
                            oob_is_err=False,
                        )
                        for st in sts:
                            tile.add_dep_helper(sc_inst.ins, st.ins,
                                                reason="scatter after store")

    nc.compile()
    return nc


def _prepare_inputs(all_memory, last_memory, item_seq, mask, U_w, W_w, V_w, E_w):
    import ml_dtypes
    bf = ml_dtypes.bfloat16
    all_memory = np.asarray(all_memory, dtype=np.float32).astype(bf)
    last_memory = np.asarray(last_memory, dtype=np.float32)
    item_seq = np.asarray(item_seq)
    mask = np.asarray(mask)
    wcat = np.ascontiguousarray(np.concatenate(
        [np.asarray(U_w, np.float32), np.asarray(W_w, np.float32),
         np.asarray(V_w, np.float32).reshape(H, 1)], axis=1).astype(bf))
    E_w = np.asarray(E_w, dtype=np.float32).astype(bf)

    # ----- host-side scatter index prep (per core, per virtual slot) -----
    # core c's out tensor has 9 row-blocks: slot 0 = physical block c (own
    # rows, stored first), slot 1+p = physical block p. Element (b, col) of
    # core c maps to row9 = b%128 if b//128==c else (1+b//128)*128 + b%128.
    items = item_seq.astype(np.int64)
    valid = items > 0
    core_of = items // NL
    b_idx = np.arange(B)[:, None].repeat(S, axis=1)
    col_in_core = items - core_of * NL                   # [B,S]

    offs = {}
    for c in range(NCORES):
        for slot in range(NCORES + 1):
            if slot == 1 + c:
                offs[(c, slot)] = np.zeros(0, np.int32)
                continue
            p = c if slot == 0 else slot - 1
            sel = valid & (core_of == c) & ((b_idx // 128) == p)
            row9 = slot * 128 + (b_idx[sel] % 128)
            offs[(c, slot)] = (row9 * NL + col_in_core[sel]).astype(np.int32)
    totals = [sum(offs[(c, s)].size for s in range(NCORES + 1))
              for c in range(NCORES)]
    n_scat = max(2, -(-max(totals) // 128))
    sidx_all = np.full((NCORES, 128, n_scat), SENTINEL, dtype=np.int32)
    dep = np.zeros((NCORES, n_scat), dtype=np.int64)
    for c in range(NCORES):
        flat = np.full(n_scat * 128, SENTINEL, dtype=np.int32)
        cbs = np.zeros(n_scat * 128, dtype=np.int64)
        pos = 0
        for slot in range(NCORES + 1):
            o = offs[(c, slot)]
            flat[pos:pos + o.size] = o
            cbs[pos:pos + o.size] = slot
            pos += o.size
        sidx_all[c] = flat.reshape(n_scat, 128).T
        dep[c] = cbs.reshape(n_scat, 128).max(axis=1)
    dep_cb = tuple(int(x) for x in dep.max(axis=0))
    scat_w = (n_scat, dep_cb)

    # multiplicative mask on exp(scores): 0 at masked positions
    maskmul = np.where(mask, np.float32(0.0), np.float32(1.0)).astype(np.float32)
    lmk = np.concatenate([last_memory.astype(np.float32), maskmul], axis=1)
    in_maps = []
    for c in range(NCORES):
        r0, r1 = c * BL, (c + 1) * BL
        in_maps.append({
            "am": np.ascontiguousarray(
                all_memory[r0:r1].reshape(BL, S * H)),
            "lmk": np.ascontiguousarray(lmk[r0:r1]),
            "wcat": wcat,
            "ew": np.ascontiguousarray(E_w[:, c * NL:(c + 1) * NL]),
            "sidx": np.ascontiguousarray(sidx_all[c]),
        })
    return scat_w, in_maps


def kernel(all_memory, last_memory, item_seq, mask, U_w, W_w, V_w, V_b, E_w):
    from concourse.bass_utils import run_bass_kernel_spmd

    scat_w, in_maps = _prepare_inputs(
        all_memory, last_memory, item_seq, mask, U_w, W_w, V_w, E_w)
    if scat_w not in _BUILT:
        _BUILT[scat_w] = _build(scat_w)
    nc = _BUILT[scat_w]
    res = run_bass_kernel_spmd(nc, in_maps, core_ids=list(range(NCORES)))
    global _LAST_RESULTS
    _LAST_RESULTS = res
    shards = []
    for c in range(NCORES):
        o9 = res.results[c]["out"]          # [B + BL, NL], 9 virtual blocks
        shard = o9[BL:].copy()              # physical blocks 0..7 (slots 1..8)
        shard[c * BL:(c + 1) * BL] = o9[0:BL]   # own block: masked copy
        shards.append(shard)
    return np.concatenate(shards, axis=1).astype(np.float32)


# revision 27
# speedup vs baseline: 1.4420x; 1.0025x over previous
"""Trainium2 Bass kernel for nn_Explore_Decoder (scatter_memory).

Full computation:
    a      = all_memory @ U_w                         [B,S,H]
    l      = (last_memory @ W_w)[:,None,:]            [B,1,H]
    scores = (tanh(a+l) @ V_w + V_b)[...,0]           [B,S]
    scores = where(mask, -1e9, scores)
    alpha  = softmax(scores, axis=1)
    out_e  = sum(alpha * all_memory, axis=1)          [B,H]
    feats  = concat([out_e, last_memory], axis=1)     [B,2H]
    logits = feats @ E_w                              [B,N]
    logits = where(seen_item, -inf, logits)           (scatter of item_seq)
    return sigmoid(logits)

Sharding (8 cores):
  Phase 1 (attention): data-parallel over B; core c owns rows [c*128,(c+1)*128).
    Online (fused) softmax: scores are tanh-bounded so exp needs no max pass;
    the mask is a 0/1 multiplier on exp; the alpha-weighted readout accumulates
    per block; 1/sum folds into one diag matmul. Produces featsT [2H,128] bf16,
    AllGather -> featsT for all B.
  Phase 2 (logits): tensor-parallel over N; core c owns cols [c*6250,(c+1)*6250),
    all in bf16 (the 2e-2 harness gate leaves plenty of room). 9 virtual row-
    blocks: slot 0 = own rows computed from local SBUF feats BEFORE the
    AllGather, so its store (and the Pool-serialized scatter stream, the
    critical path: ~100 indirect DMAs x ~1.04us SWDGE each) starts early.
    Scatter: host-precomputed flat offsets of seen items; one indirect DMA per
    128 offsets writes 0.0 ( = sigmoid(-inf)) over the dense output.
"""

import numpy as np

B, S, H, N = 1024, 100, 128, 50000
NCORES = 8
BL = B // NCORES          # 128 batch rows per core (phase 1)
NL = N // NCORES          # 6250 vocab cols per core (phase 2)
H2 = 2 * H
SB = 4                    # s-values per phase-1 block
SENTINEL = np.int32(2**30)

_BUILT = {}               # scat_w -> compiled Bass module
_LAST_RESULTS = None      # BassKernelResults of the most recent run (for tests)


def _default_spec(n_scat=100):
    # representative scatter spec for profiling: uniform item distribution
    per = 12800 / NCORES
    dep = tuple(min(NCORES - 1, int((w + 1) * 128 / per)) for w in range(n_scat))
    return (n_scat, dep)


def _build(scat_w, reps: int = 1, timeline: bool = False,
           no_scatter: bool = False):
    # scat_w: (n_scat, dep_cb) from host packing
    import concourse.bass as bass
    import concourse.mybir as mybir
    import concourse.tile as tile
    from concourse import bacc
    from concourse.masks import make_identity

    f32 = mybir.dt.float32
    bf16 = mybir.dt.bfloat16
    i32 = mybir.dt.int32
    AF = mybir.ActivationFunctionType
    ALU = mybir.AluOpType
    AX = mybir.AxisListType

    n_scat, dep_cb = scat_w

    nc = bacc.Bacc(None, target_bir_lowering=False, debug=False)

    am = nc.dram_tensor("am", [BL, S * H], bf16, kind="ExternalInput")
    # lmk = [last_memory | maskmul] rows; wcat = [U_w | W_w | V_w] columns
    lmk = nc.dram_tensor("lmk", [BL, H + S], f32, kind="ExternalInput")
    wcat = nc.dram_tensor("wcat", [H, H + H + 1], bf16, kind="ExternalInput")
    ew = nc.dram_tensor("ew", [H2, NL], bf16, kind="ExternalInput")
    # [p, w] = flat element offset (into the [B, NL] output) for partition p of
    # indirect-DMA instruction w; instruction w may fire once the store of
    # row-block dep_cb[w] has landed. SENTINEL = unused slot.
    sidx = nc.dram_tensor("sidx", [128, n_scat], i32, kind="ExternalInput")
    # 9 virtual row-blocks: slot 0 = this core's own rows (computed from local
    # SBUF feats BEFORE the AllGather so its store + scatters start early);
    # slots 1..8 = gathered blocks 0..7 (slot 1+c duplicates slot 0 unmasked;
    # the host drops it at unshard).
    out = nc.dram_tensor("out", [B + BL, NL], bf16, kind="ExternalOutput")
    # timeline mode: single-core cost-model sim can't price collectives or
    # full-tensor indirect APs; swap in traffic-equivalent stand-ins
    dumout = nc.dram_tensor("dumout", [128, 1], bf16) if timeline else None

    with tile.TileContext(nc) as tc:
      for _rep in range(reps):
        with (
            tc.tile_pool(name="consts", bufs=1) as cp,
            tc.tile_pool(name="amp", bufs=1) as amp,
            tc.tile_pool(name="ewp", bufs=1) as ewp,
            tc.tile_pool(name="dram", bufs=1, space="DRAM") as dp,
            tc.tile_pool(name="smax", bufs=1) as sm,
        ):
            # two batched const DMAs first (lm unblocks the in-order PE
            # queue's first transpose), then the am stream, then ew, then sidx
            ident = cp.tile([128, 128], f32)
            make_identity(nc, ident[:])
            ident_bf = cp.tile([128, 128], bf16)
            nc.vector.tensor_copy(ident_bf[:], ident[:])
            lmk_sb = cp.tile([BL, H + S], f32)
            nc.sync.dma_start(out=lmk_sb[:], in_=lmk[:, :])
            lm_sb = lmk_sb[:, 0:H]
            maskb_sb = lmk_sb[:, H:H + S]
            wcat_sb = cp.tile([H, H + H + 1], bf16)
            nc.sync.dma_start(out=wcat_sb[:], in_=wcat[:, :])
            uw_sb = wcat_sb[:, 0:H]
            ww_sb = wcat_sb[:, H:2 * H]
            vw_sb = wcat_sb[:, 2 * H:2 * H + 1]
            zeros_sb = cp.tile([128, 1], bf16)
            nc.vector.memset(zeros_sb[:], 0.0)

            am_t = amp.tile([BL, S * H], bf16)
            AMCH = 10 * H
            for a0 in range(0, S * H, AMCH):
                nc.sync.dma_start(out=am_t[:, a0:a0 + AMCH],
                                  in_=am[:, a0:a0 + AMCH])
            amv = am_t[:].rearrange("p (s h) -> p s h", h=H)

            # phase-2 weights: no deps, needed only ~20us in
            ew_top = ewp.tile([H, NL], bf16)
            nc.sync.dma_start(out=ew_top[:], in_=ew[0:H, :])
            ew_bot = ewp.tile([H, NL], bf16)
            nc.sync.dma_start(out=ew_bot[:], in_=ew[H:H2, :])
            six_all = cp.tile([128, n_scat], i32)
            nc.sync.dma_start(out=six_all[:], in_=sidx[:, :])

            feats_local = dp.tile([H2, BL], bf16)
            gath = dp.tile([NCORES * H2, BL], bf16)

            # ---------------- Phase 1: attention over S, rows of this core ----
            with (
                tc.tile_pool(name="ps_t", bufs=3, space="PSUM") as ps_t,
                tc.tile_pool(name="ps_z", bufs=2, space="PSUM") as ps_z,
                tc.tile_pool(name="ps_acc", bufs=1, space="PSUM") as ps_acc,
                tc.tile_pool(name="xtp", bufs=6) as xtp,
                tc.tile_pool(name="tzp", bufs=6) as tzp,
            ):
                # last_memory^T  [H, BL] , replicated x SB for the Z matmul rhs
                lmT_ps = ps_t.tile([128, 512], f32, tag="tps")
                nc.tensor.transpose(out=lmT_ps[:, :H], in_=lm_sb,
                                    identity=ident[:])
                lmT_sb = cp.tile([H, BL], f32)
                nc.vector.tensor_copy(lmT_sb[:], lmT_ps[:, :H])
                lmT_rep = cp.tile([H, SB * BL], bf16)
                nc.vector.tensor_copy(
                    lmT_rep[:].rearrange("h (s b) -> h s b", s=SB),
                    lmT_sb[:].unsqueeze(1).broadcast_to([H, SB, BL]),
                )
                # feats rows H..2H = last_memory^T (raw), bf16
                lmT_bf = cp.tile([H, BL], bf16)
                nc.vector.tensor_copy(lmT_bf[:], lmT_sb[:])
                nc.sync.dma_start(out=feats_local[H:H2, :], in_=lmT_bf[:])

                # Online (fused) attention: scores are tanh-bounded
                # (|sc| <= ||V||_1 ~ 2.6), so exp without max-subtraction is
                # safe; the mask is a 0/1 multiplier on exp(sc); the
                # alpha-weighted readout accumulates per block inside the
                # loop; 1/sum folds into one diag matmul at the end.
                sc_ps = ps_acc.tile([BL, S], f32, tag="sc")
                oe_ps = ps_acc.tile([H, BL], f32, tag="oe")
                alpham = sm.tile([BL, S], f32)   # masked exp(sc), unnormalized
                with tc.tile_pool(name="wamp", bufs=4) as wamp:
                  for bi, s0 in enumerate(range(0, S, SB)):
                    sw = min(SB, S - s0)
                    sl = slice(s0, s0 + sw)
                    xt_ps = ps_t.tile([128, SB * 128], bf16, tag="tps")
                    for j in range(sw):
                        nc.tensor.transpose(
                            out=xt_ps[:, j * 128:(j + 1) * 128],
                            in_=amv[:, s0 + j, :],
                            identity=ident_bf[:],
                        )
                    xt = xtp.tile([128, SB * 128], bf16)
                    if bi % 2 == 0:
                        nc.vector.tensor_copy(xt[:, :sw * 128],
                                              xt_ps[:, :sw * 128])
                    else:
                        nc.scalar.copy(xt[:, :sw * 128], xt_ps[:, :sw * 128])
                    z_ps = ps_z.tile([128, SB * BL], f32)
                    nc.tensor.matmul(z_ps[:, :sw * BL], lhsT=uw_sb,
                                     rhs=xt[:, :sw * 128],
                                     start=True, stop=False)
                    nc.tensor.matmul(z_ps[:, :sw * BL], lhsT=ww_sb,
                                     rhs=lmT_rep[:, :sw * BL],
                                     start=False, stop=True)
                    tz = tzp.tile([128, SB * BL], bf16)
                    nc.scalar.activation(tz[:, :sw * BL], z_ps[:, :sw * BL],
                                         AF.Tanh)
                    for j in range(sw):
                        s = s0 + j
                        nc.tensor.matmul(
                            sc_ps[:, s:s + 1],
                            lhsT=tz[:, j * 128:(j + 1) * 128],
                            rhs=vw_sb,
                            start=True, stop=True,
                        )
                    # unnormalized masked weights for this block
                    nc.scalar.activation(alpham[:, sl], sc_ps[:, sl], AF.Exp)
                    nc.vector.tensor_tensor(alpham[:, sl], alpham[:, sl],
                                            maskb_sb[:, sl], op=ALU.mult)
                    wam = wamp.tile([BL, SB * H], f32)
                    wv = wam[:].rearrange("p (s h) -> p s h", h=H)
                    nc.vector.tensor_tensor(
                        wv[:, :sw, :],
                        amv[:, sl, :],
                        alpham[:, sl].unsqueeze(2).broadcast_to([BL, sw, H]),
                        op=ALU.mult,
                    )
                    for j in range(sw):
                        s = s0 + j
                        nc.tensor.matmul(oe_ps[:], lhsT=wv[:, j, :],
                                         rhs=ident[:],
                                         start=(s == 0), stop=(s == S - 1),
                                         is_transpose=True)

                # normalization: oeT = O^T @ diag(1/sum)
                sum_sb = sm.tile([BL, 1], f32)
                nc.vector.tensor_reduce(sum_sb[:], alpham[:], AX.X, op=ALU.add)
                rsum = sm.tile([BL, 1], f32)
                nc.vector.reciprocal(rsum[:], sum_sb[:])
                # preload the Sigmoid act table (keeps the table switch off
                # slot-0's critical path)
                sigwarm = sm.tile([BL, 1], f32)
                nc.scalar.activation(sigwarm[:], rsum[:], AF.Sigmoid)
                rdiag = sm.tile([BL, BL], bf16)
                nc.vector.tensor_scalar_mul(rdiag[:], ident_bf[:],
                                            rsum[:, 0:1])
                OT_sb = sm.tile([H, BL], f32)
                nc.vector.tensor_copy(OT_sb[:], oe_ps[:])
                O_ps = ps_t.tile([128, 512], f32, tag="tps")
                nc.tensor.transpose(out=O_ps[:, :H], in_=OT_sb[:],
                                    identity=ident[:])
                O_sb = sm.tile([BL, H], bf16)
                nc.vector.tensor_copy(O_sb[:], O_ps[:, :H])
                oeT_ps = ps_acc.tile([H, BL], f32, tag="sc")
                nc.tensor.matmul(oeT_ps[:], lhsT=O_sb[:], rhs=rdiag[:],
                                 start=True, stop=True)
                oeT_sb = sm.tile([H, BL], bf16)
                nc.vector.tensor_copy(oeT_sb[:], oeT_ps[:])
                nc.sync.dma_start(out=feats_local[0:H, :], in_=oeT_sb[:])

            # ---------------- AllGather feats ----------------
            if timeline:
                gvw = gath[:].rearrange("(c f) b -> c f b", f=H2)
                nc.sync.dma_start(
                    out=gvw[:, :, :],
                    in_=feats_local[:].unsqueeze(0).broadcast_to(
                        [NCORES, H2, BL]))
            else:
                nc.gpsimd.collective_compute(
                    "AllGather",
                    mybir.AluOpType.bypass,
                    replica_groups=[list(range(NCORES))],
                    ins=[feats_local[:].opt()],
                    outs=[gath[:].opt()],
                )

            # ---------------- Phase 2: logits + sigmoid over local cols -------
            if timeline:
                out_flat = dumout[:, :].rearrange("a b -> (a b)").unsqueeze(1)
            else:
                out_flat = out[:, :].rearrange("a b -> (a b)").unsqueeze(1)

            def scat_ap(k):
                # unique fake dep region per scatter: suppresses Tile's
                # WAW serialization between scatters (all write 0.0 at
                # host-guaranteed positions; order among them is free).
                # The real store->scatter ordering is added explicitly.
                return bass.AP(
                    tensor=out_flat.tensor, offset=0, ap=out_flat.ap,
                    dep_tracking_offset=(1 << 33) + k * (1 << 23))

            with (
                tc.tile_pool(name="ps2", bufs=3, space="PSUM") as ps2,
                tc.tile_pool(name="pswarm", bufs=1, space="PSUM") as pswarm,
                tc.tile_pool(name="outp", bufs=3) as outp,
                tc.tile_pool(name="gp", bufs=3) as gp,
            ):
                NCHW = 1024
                scat_by_dep = {}
                for w, d in enumerate(dep_cb):
                    scat_by_dep.setdefault(d, []).append(w)
                for cb in range(NCORES + 1):
                    if cb == 0:
                        g_oe = oeT_sb[:]
                        g_lm = lmT_bf[:]
                    else:
                        if cb == 1:
                            # slots 1..8 stall on the AllGather; keep the PE
                            # p-state ramp hot with filler transposes
                            warm_ps = pswarm.tile([128, 128], f32)
                            for _ in range(40):
                                nc.tensor.transpose(out=warm_ps[:],
                                                    in_=ident[:],
                                                    identity=ident[:])
                        p = cb - 1
                        # per-slot feats load: [h, (oe|lm), b] as [128, 256]
                        g_slot = gp.tile([128, 2 * 128], bf16)
                        nc.sync.dma_start(
                            out=g_slot[:],
                            in_=gath[p * H2:(p + 1) * H2, :].rearrange(
                                "(t p2) b -> p2 t b", p2=128))
                        g_oe = g_slot[:, 0:128]
                        g_lm = g_slot[:, 128:256]
                    out_sb = outp.tile([128, NL], bf16)
                    sts = []
                    for n0 in range(0, NL, NCHW):
                        w = min(NCHW, NL - n0)
                        pt = ps2.tile([128, NCHW], f32)
                        for q0 in range(0, w, 512):
                            qw = min(512, w - q0)
                            sl = slice(n0 + q0, n0 + q0 + qw)
                            po = pt[:, q0:q0 + qw]
                            nc.tensor.matmul(po, lhsT=g_oe,
                                             rhs=ew_top[:, sl],
                                             start=True, stop=False)
                            nc.tensor.matmul(po, lhsT=g_lm,
                                             rhs=ew_bot[:, sl],
                                             start=False, stop=True)
                        nc.scalar.activation(out_sb[:, n0:n0 + w], pt[:, :w],
                                             AF.Sigmoid)
                        # store per chunk: the slot's last store (and thus its
                        # scatters) completes ~1 chunk after the last sigmoid
                        sts.append(nc.sync.dma_start(
                            out=out[cb * 128:(cb + 1) * 128, n0:n0 + w],
                            in_=out_sb[:, n0:n0 + w]))
                    # seen-item mask: scatter 0.0 over stored rows. HW indirect
                    # DMA consumes one offset per partition -> 128 single-
                    # element writes per instruction; instruction w carries
                    # offsets only from row-blocks <= dep_cb[w].
                    for w in ([] if no_scatter else scat_by_dep.get(cb, [])):
                        sc_inst = nc.gpsimd.indirect_dma_start(
                            out=scat_ap(w),
                            out_offset=bass.IndirectOffsetOnAxis(
                                ap=six_all[:, w:w + 1], axis=0),
                            in_=zeros_sb[:, :],
                            in_offset=None,
                            bounds_check=(127 if timeline else (B + BL) * NL - 1),
                            oob_is_err=False,
                        )
                        for st in sts:
                            tile.add_dep_helper(sc_inst.ins, st.ins,
                                                reason="scatter after store")

    nc.compile()
    return nc


def _prepare_inputs(all_memory, last_memory, item_seq, mask, U_w, W_w, V_w, E_w):
    import ml_dtypes
    bf = ml_dtypes.bfloat16
    all_memory = np.asarray(all_memory, dtype=np.float32).astype(bf)
    last_memory = np.asarray(last_memory, dtype=np.float32)
    item_seq = np.asarray(item_seq)
    mask = np.asarray(mask)
    wcat = np.ascontiguousarray(np.concatenate(
        [np.asarray(U_w, np.float32), np.asarray(W_w, np.float32),
         np.asarray(V_w, np.float32).reshape(H, 1)], axis=1).astype(bf))
    E_w = np.asarray(E_w, dtype=np.float32).astype(bf)

    # ----- host-side scatter index prep (per core, per virtual slot) -----
    # core c's out tensor has 9 row-blocks: slot 0 = physical block c (own
    # rows, stored first), slot 1+p = physical block p. Element (b, col) of
    # core c maps to row9 = b%128 if b//128==c else (1+b//128)*128 + b%128.
    items = item_seq.astype(np.int64)
    valid = items > 0
    core_of = items // NL
    b_idx = np.arange(B)[:, None].repeat(S, axis=1)
    col_in_core = items - core_of * NL                   # [B,S]

    offs = {}
    for c in range(NCORES):
        for slot in range(NCORES + 1):
            if slot == 1 + c:
                offs[(c, slot)] = np.zeros(0, np.int32)
                continue
            p = c if slot == 0 else slot - 1
            sel = valid & (core_of == c) & ((b_idx // 128) == p)
            row9 = slot * 128 + (b_idx[sel] % 128)
            offs[(c, slot)] = (row9 * NL + col_in_core[sel]).astype(np.int32)
    totals = [sum(offs[(c, s)].size for s in range(NCORES + 1))
              for c in range(NCORES)]
    n_scat = max(2, -(-max(totals) // 128))
    sidx_all = np.full((NCORES, 128, n_scat), SENTINEL, dtype=np.int32)
    dep = np.zeros((NCORES, n_scat), dtype=np.int64)
    for c in range(NCORES):
        flat = np.full(n_scat * 128, SENTINEL, dtype=np.int32)
        cbs = np.zeros(n_scat * 128, dtype=np.int64)
        pos = 0
        for slot in range(NCORES + 1):
            o = offs[(c, slot)]
            flat[pos:pos + o.size] = o
            cbs[pos:pos + o.size] = slot
            pos += o.size
        sidx_all[c] = flat.reshape(n_scat, 128).T
        dep[c] = cbs.reshape(n_scat, 128).max(axis=1)
    dep_cb = tuple(int(x) for x in dep.max(axis=0))
    scat_w = (n_scat, dep_cb)

    # multiplicative mask on exp(scores): 0 at masked positions
    maskmul = np.where(mask, np.float32(0.0), np.float32(1.0)).astype(np.float32)
    lmk = np.concatenate([last_memory.astype(np.float32), maskmul], axis=1)
    in_maps = []
    for c in range(NCORES):
        r0, r1 = c * BL, (c + 1) * BL
        in_maps.append({
            "am": np.ascontiguousarray(
                all_memory[r0:r1].reshape(BL, S * H)),
            "lmk": np.ascontiguousarray(lmk[r0:r1]),
            "wcat": wcat,
            "ew": np.ascontiguousarray(E_w[:, c * NL:(c + 1) * NL]),
            "sidx": np.ascontiguousarray(sidx_all[c]),
        })
    return scat_w, in_maps


def kernel(all_memory, last_memory, item_seq, mask, U_w, W_w, V_w, V_b, E_w):
    from concourse.bass_utils import run_bass_kernel_spmd

    scat_w, in_maps = _prepare_inputs(
        all_memory, last_memory, item_seq, mask, U_w, W_w, V_w, E_w)
    if scat_w not in _BUILT:
        _BUILT[scat_w] = _build(scat_w)
    nc = _BUILT[scat_w]
    res = run_bass_kernel_spmd(nc, in_maps, core_ids=list(range(NCORES)))
    global _LAST_RESULTS
    _LAST_RESULTS = res
    shards = []
    for c in range(NCORES):
        o9 = res.results[c]["out"]          # [B + BL, NL], 9 virtual blocks
        shard = o9[BL:].copy()              # physical blocks 0..7 (slots 1..8)
        shard[c * BL:(c + 1) * BL] = o9[0:BL]   # own block: masked copy
        shards.append(shard)
    return np.concatenate(shards, axis=1).astype(np.float32)
